# revision 9
# baseline (speedup 1.0000x reference)
"""DiagonalLSTM Trainium2 kernel.

Reference computation (per batch element b):
  xs = skew(x)                               # (Cin, H, 2W-1), row r shifted right by r
  z_is = w_is @ xs + b_is                    # 1x1 conv -> 4*HID channels
  for t in 0..2W-2:                          # sequential scan over skewed width
      hs[o, r] = wss[o,c,0] h[c,r-1] + wss[o,c,1] h[c,r] + b_ss[o]
      z = z_is[:, :, t] + hs
      i, f, o_, g = sig, sig, sig, tanh of the 4 gate quarters
      c = f*c + i*g ; h = o_*tanh(c)
  out = unskew(h history)

Sharding: data-parallel over batch B=8 across the 8 NeuronCores (the t-scan is
inherently sequential; each core runs its own batch element's full scan).

Per-core layout (128 partitions = channels), gate order [f, i, g, o]:
 - gates-on-partitions: per step the gates live in THREE psum banks -- f
   (128x64), i|g (128x128), o (128x64) -- because Tile's bank-overlap tracker
   serializes any read of a bank behind ALL matmul writes to it; separate
   banks let sigmoid(f) start after just f's two recurrent taps, so the
   t2 = sig_f * c vector op overlaps the remaining activations.
 - g is computed VIA SIGMOID: tanh(z) = 2*sigmoid(2z) - 1, the factor 2 folded
   into the g columns of all weights host-side; the -0.5 / *2 corrections fuse
   into scalar_tensor_tensor ops:
       t2 = sig_f * c ;  t1 = (sig_g - 0.5) * sig_i  # = i*g/2
       c  = t1*2 + t2
 - z_is is NOT precomputed: per step it is one fp16 matmul per gate directly
   into the step's psum banks, emitted LOOKAHEAD steps early so the PE does it
   while waiting for the recurrent h. The recurrent taps accumulate on top.
   NOTE start=True clears has_written for the WHOLE psum bank, so only the
   first matmul per bank-round carries it (order pinned with explicit deps).
 - x is pre-skewed and pre-cast to fp16 ON THE HOST, t-major: xs[c, t*64+r]
   (zero padded), so the step-t rhs is the contiguous slice xs[:, t*64:(t+1)*64]
   with out-of-band rows exact zeros, and the DMA streams in 16-step chunks so
   the scan starts after the first chunk. No on-device memset/cast.
 - h is written as fp16 into a (128 x 66) rhs buffer with cols 0:2 always 0 and
   h at cols 2:66 (4-byte aligned => the all-fp16 h-mul gets the DVE 2x mode);
   tap0 (h[r-1]) = cols 1:65 and tap1 (h[r]) = cols 2:66 of the same buffer.
 - sigma_o and tanh(c) are produced as fp16 (only consumed by the h products);
   the f/i/g sigmoids and the c accumulator stay fp32.
 - h history is stored fp32 directly in unskewed layout hist[c, r*64+w] via a
   stride-63 write of the in-band rows (t = r+w  =>  flat = r*63 + t); the
   output DMA is chunked by row groups overlapping the scan tail.
 - zero-bias fast path: every per-step op covers only the ACTIVE row window
   [max(0,t-63) & ~1, min(t,63)] -- below-diagonal rows are exactly 0 and rows
   with t-r > 63 are dead, so on average half the free-dim work disappears
   (~1.9us/step vs ~2.16). The nonzero-bias path keeps the full width.
"""

import sys

if "/opt/trn_rl_repo" not in sys.path:
    sys.path.insert(0, "/opt/trn_rl_repo")

import numpy as np

N_CORES = 8
HID = 128
CIN = 128
H = 64
W = 64
T = 2 * W - 1  # 127
LOOKAHEAD = 1
RCHUNK = 16  # epilogue row-chunk size

_PROGRAM_CACHE = {}


def _build_program(use_bias: bool):
    import concourse.bacc as bacc
    import concourse.tile as tile
    from concourse import mybir

    fp32 = mybir.dt.float32
    fp16 = mybir.dt.float16
    AFT = mybir.ActivationFunctionType
    ALU = mybir.AluOpType

    nc = bacc.Bacc("TRN2", debug=False, num_devices=N_CORES)
    xs_d = nc.dram_tensor("xs", [CIN, T * 64], fp16, kind="ExternalInput")
    wis_d = nc.dram_tensor("wis", [CIN, 4 * HID], fp16, kind="ExternalInput")
    wss0_d = nc.dram_tensor("wss0", [HID, 4 * HID], fp16, kind="ExternalInput")
    wss1_d = nc.dram_tensor("wss1", [HID, 4 * HID], fp16, kind="ExternalInput")
    bias_d = nc.dram_tensor("bias", [HID, 4], fp32, kind="ExternalInput")
    out_d = nc.dram_tensor("out", [HID, H * W], fp32, kind="ExternalOutput")

    with tile.TileContext(nc) as tc:
        with (
            tc.tile_pool(name="persist", bufs=1) as pp,
            tc.tile_pool(name="gates", bufs=3) as gp,
            tc.tile_pool(name="psf", bufs=2, space="PSUM") as psf,
            tc.tile_pool(name="psc", bufs=1, space="PSUM") as psc,
            tc.tile_pool(name="psig", bufs=3, space="PSUM") as psig,
            tc.tile_pool(name="pso", bufs=2, space="PSUM") as pso,
        ):
            xskew = pp.tile([128, T * 64], fp16, tag="xskew")
            wis_s = pp.tile([128, 512], fp16, tag="wis")
            wss0_s = pp.tile([128, 512], fp16, tag="wss0")
            wss1_s = pp.tile([128, 512], fp16, tag="wss1")
            bias_s = pp.tile([128, 4], fp32, tag="bias")
            tread = pp.tile([128, 704], fp16, tag="tread")
            rhs = [
                pp.tile([128, 66], fp16, tag=f"rhs{i}", name=f"rhs{i}")
                for i in range(2)
            ]
            cbuf = psc.tile([128, 64], fp32, tag="cbuf")
            hist = pp.tile([128, H * W], fp32, tag="hist")
            warm = pp.tile([128, 1], fp32, tag="warm")

            # --- prologue ---
            # xs chunks stream on gpsimd while the (small) weight DMAs issue
            # in parallel from the scalar engine; the scan starts as soon as
            # chunk 0 + weights land.
            for k in range(0, T, 16):
                hi = min(T, k + 16) * 64
                nc.gpsimd.dma_start(out=xskew[:, k * 64 : hi], in_=xs_d.ap()[:, k * 64 : hi])
            nc.scalar.dma_start(out=wis_s, in_=wis_d.ap())
            nc.sync.dma_start(out=wss0_s, in_=wss0_d.ap())
            nc.sync.dma_start(out=wss1_s, in_=wss1_d.ap())
            nc.scalar.dma_start(out=bias_s, in_=bias_d.ap())

            # Pull the sigmoid ACT table load to the start (overlaps DMA).
            # The kernel is ALL-sigmoid (tanh = 2*sig(2x)-1 everywhere): an
            # ACT func switch costs ~110ns on the switching instruction, so
            # never switching saves ~220ns/step off the critical chain.
            nc.vector.memset(warm, 0.0)
            nc.scalar.activation(warm, warm, AFT.Sigmoid)

            nc.vector.memset(rhs[0], 0.0)
            nc.vector.memset(rhs[1], 0.0)
            nc.vector.memset(cbuf, 0.0)
            nc.vector.memset(tread, 0.0)

            def win(t):
                # active row window: below-diagonal rows are exactly 0 (zero
                # bias) and rows with t-r > 63 are dead, so ops only cover
                # [r0e, r1]. r0 rounded down to even keeps the fp16 h-write
                # 4B-aligned (the extra row is dead). Bias path: full width.
                if use_bias:
                    return 0, 63
                r0 = 0 if t < 64 else t - 63
                r1 = t if t < 63 else 63
                return r0 & ~1, r1

            pf = [None] * T
            pig = [None] * T
            po = [None] * T

            def emit_z(t):
                pf[t] = psf.tile([128, 64], fp32, tag="pf", name=f"pf{t}")
                pig[t] = psig.tile([128, 128], fp32, tag="pig", name=f"pig{t}")
                po[t] = pso.tile([128, 64], fp32, tag="po", name=f"po{t}")
                a, b = win(t)
                r = xskew[:, t * 64 + a : t * 64 + b + 1]
                nc.tensor.matmul(pf[t][:, a : b + 1], lhsT=wis_s[:, 0:128], rhs=r,
                                 start=True, stop=False, skip_group_check=True)
                mi = nc.tensor.matmul(pig[t][:, a : b + 1], lhsT=wis_s[:, 128:256], rhs=r,
                                      start=True, stop=False, skip_group_check=True)
                mg = nc.tensor.matmul(pig[t][:, 64 + a : 64 + b + 1], lhsT=wis_s[:, 256:384], rhs=r,
                                      start=False, stop=False, skip_group_check=True)
                tile.add_dep_helper(mg.ins, mi.ins, sync=False,
                                    reason="bank-clear MM must run first")
                nc.tensor.matmul(po[t][:, a : b + 1], lhsT=wis_s[:, 384:512], rhs=r,
                                 start=True, stop=False, skip_group_check=True)

            for t in range(LOOKAHEAD):
                emit_z(t)

            # --- the 127-step scan (gate order: f, i, g, o) ---
            for t in range(T):
                if t + LOOKAHEAD < T:
                    emit_z(t + LOOKAHEAD)

                a, b = win(t)
                n = b - a + 1
                rbuf = rhs[t % 2]
                tap0 = rbuf[:, 1 + a : 2 + b]
                tap1 = rbuf[:, 2 + a : 3 + b]

                def rec(dst, q, stop):
                    nc.tensor.matmul(dst, lhsT=wss0_s[:, q * 128 : (q + 1) * 128], rhs=tap0,
                                     start=False, stop=False, skip_group_check=True)
                    nc.tensor.matmul(dst, lhsT=wss1_s[:, q * 128 : (q + 1) * 128], rhs=tap1,
                                     start=False, stop=stop, skip_group_check=True)

                rec(pf[t][:, a : b + 1], 0, True)             # f first
                rec(pig[t][:, a : b + 1], 1, False)           # i
                rec(pig[t][:, 64 + a : 64 + b + 1], 2, True)  # g
                rec(po[t][:, a : b + 1], 3, True)             # o last

                sig = gp.tile([128, 192], fp32, tag="sig")
                so = gp.tile([128, 64], fp16, tag="so")
                if use_bias:
                    nc.scalar.activation(sig[:, 0:64], pf[t], AFT.Sigmoid, bias=bias_s[:, 0:1])
                    nc.scalar.activation(sig[:, 64:128], pig[t][:, 0:64], AFT.Sigmoid, bias=bias_s[:, 1:2])
                    nc.scalar.activation(sig[:, 128:192], pig[t][:, 64:128], AFT.Sigmoid, bias=bias_s[:, 2:3])
                else:
                    nc.scalar.activation(sig[:, a : b + 1], pf[t][:, a : b + 1], AFT.Sigmoid)
                    nc.scalar.activation(
                        sig[:, 64:192].rearrange("p (g r) -> p g r", g=2)[:, :, a : b + 1],
                        pig[t].rearrange("p (g r) -> p g r", g=2)[:, :, a : b + 1],
                        AFT.Sigmoid,
                    )

                t1 = gp.tile([128, 64], fp32, tag="t1")
                t2 = gp.tile([128, 64], fp32, tag="t2")
                # t2 = sig_f * c ; t1 = (sig_g - 0.5) * sig_i = i*g/2
                nc.vector.tensor_mul(t2[:, a : b + 1], sig[:, a : b + 1], cbuf[:, a : b + 1])
                t1_op = nc.vector.scalar_tensor_tensor(
                    t1[:, a : b + 1], sig[:, 128 + a : 128 + b + 1], -0.5,
                    sig[:, 64 + a : 64 + b + 1], ALU.add, ALU.mult
                )
                # sig_o AFTER t1 (fake dep): an ACT dispatched from idle pays a
                # ~110ns cold-start; delaying sig_o keeps the ACT queue busy
                # right up to when sig(2c)'s input (c) lands, so sig(2c) and
                # (via the treadmill below) next step's sig_f dispatch hot.
                if use_bias:
                    so_op = nc.scalar.activation(so, po[t], AFT.Sigmoid, bias=bias_s[:, 3:4])
                else:
                    so_op = nc.scalar.activation(so[:, a : b + 1], po[t][:, a : b + 1], AFT.Sigmoid)
                tile.add_dep_helper(so_op.ins, t1_op.ins, sync=True,
                                    reason="delay sig_o to keep ACT hot")
                # c = t1*2 + t2
                nc.vector.scalar_tensor_tensor(
                    cbuf[:, a : b + 1], t1[:, a : b + 1], 2.0, t2[:, a : b + 1],
                    ALU.mult, ALU.add
                )

                # tanh(c) via sigmoid: tc_s = sig(2c); tanh(c) = 2*tc_s - 1.
                # The device computes h/2 = (tc_s - 0.5) * sig_o everywhere:
                # the taps absorb the 1/2 via host-doubled wss, and the host
                # doubles the final output once after the gather.
                tc_s = gp.tile([128, 64], fp16, tag="tc")
                nc.scalar.activation(tc_s[:, a : b + 1], cbuf[:, a : b + 1], AFT.Sigmoid, scale=2.0)

                # treadmill: dep-free ACT sized to keep the scalar engine busy
                # until next step's sig_f input is ready, so sig_f goes hot.
                if t + 1 < T:
                    nB = min(446 + (6 * n) // 5, 700)
                    nc.scalar.activation(tread[:, 0:nB], tread[:, 0:nB], AFT.Sigmoid)

                # h/2 (fp16) into the next rhs buffer -- this is the serial chain
                nbuf = rhs[(t + 1) % 2]
                nc.vector.scalar_tensor_tensor(
                    nbuf[:, 2 + a : 3 + b], tc_s[:, a : b + 1], -0.5, so[:, a : b + 1],
                    ALU.add, ALU.mult
                )

                # h/2 (fp32) into unskewed history, in-band rows only (off chain)
                r0 = 0 if t < W else t - (W - 1)
                r1 = t if t < W else W - 1
                cnt = r1 - r0 + 1
                base = r0 * 63 + t
                hview = (
                    hist[:, base : base + (cnt - 1) * 63 + 1 : 63]
                    if cnt > 1
                    else hist[:, base : base + 1]
                )
                nc.vector.scalar_tensor_tensor(
                    hview, tc_s[:, r0 : r0 + cnt], -0.5, so[:, r0 : r0 + cnt],
                    ALU.add, ALU.mult
                )

                # epilogue overlap: rows [k0, k1) are final after step k1-1+63;
                # finer chunks near the end shrink the post-scan DMA tail
                for k0, k1 in ((0, 16), (16, 32), (32, 48), (48, 56), (56, 60), (60, 64)):
                    if t == k1 - 1 + 63:
                        nc.gpsimd.dma_start(
                            out=out_d.ap()[:, k0 * 64 : k1 * 64],
                            in_=hist[:, k0 * 64 : k1 * 64],
                        )

    nc.compile()
    return nc


def _get_program(use_bias: bool):
    if use_bias not in _PROGRAM_CACHE:
        _PROGRAM_CACHE[use_bias] = _build_program(use_bias)
    return _PROGRAM_CACHE[use_bias]


def _prep_weights(w):
    """(512, 128) -> (128, 512) fp16 with gate column order [f, i, 2g, o]."""
    wt = w.T.astype(np.float32)  # (128, 512) in [i, f, o, g] order
    out = np.concatenate(
        [wt[:, 128:256], wt[:, 0:128], 2.0 * wt[:, 384:512], wt[:, 256:384]], axis=1
    )
    return np.ascontiguousarray(out.astype(np.float16))


def kernel(x, w_is, b_is, w_ss, b_ss, _trace=False, _trace_kwargs=None):
    from concourse.bass_utils import run_bass_kernel_spmd

    x = np.asarray(x, dtype=np.float32)
    w_is = np.asarray(w_is, dtype=np.float32)
    b_is = np.asarray(b_is, dtype=np.float32)
    w_ss = np.asarray(w_ss, dtype=np.float32)
    b_ss = np.asarray(b_ss, dtype=np.float32)
    B = x.shape[0]
    assert x.shape == (B, CIN, H, W), x.shape

    bias = (b_is + b_ss).astype(np.float32)  # (512,) in [i, f, o, g] order
    use_bias = bool(np.any(bias != 0.0))
    nc = _get_program(use_bias)

    wis_h = _prep_weights(w_is)
    # the device's h is h/2 (tanh via sigmoid) -> recurrent weights x2
    wss0_h = _prep_weights(2.0 * w_ss[:, :, 0, 0])
    wss1_h = _prep_weights(2.0 * w_ss[:, :, 1, 0])
    bq = bias.reshape(4, HID)  # [i, f, o, g]
    bias_h = np.ascontiguousarray(
        np.stack([bq[1], bq[0], 2.0 * bq[3], bq[2]], axis=1).astype(np.float32)
    )  # (128, 4) in [f, i, 2g, o] order

    # host-side skew + fp16 cast, t-major: xs[b, c, t*64 + r] = x[b, c, r, t-r]
    xs_all = np.zeros((B, CIN, T, 64), np.float16)
    x16 = x.astype(np.float16)
    for r in range(H):
        xs_all[:, :, r : r + W, r] = x16[:, :, r, :].transpose(0, 1, 2)
    xs_all = xs_all.reshape(B, CIN, T * 64)

    in_maps = []
    for b in range(N_CORES):
        in_maps.append(
            {
                "xs": np.ascontiguousarray(xs_all[b % B]),
                "wis": wis_h,
                "wss0": wss0_h,
                "wss1": wss1_h,
                "bias": bias_h,
            }
        )

    res = run_bass_kernel_spmd(
        nc,
        in_maps,
        core_ids=list(range(N_CORES)),
        trace=_trace,
        **(_trace_kwargs or {}),
    )
    out = np.stack(
        [res.results[b]["out"].reshape(HID, H, W) for b in range(B)], axis=0
    ).astype(np.float32)
    out *= 2.0  # device history holds h/2
    if _trace:
        return out, res
    return out

